# revision 13
# baseline (speedup 1.0000x reference)
"""GCN + LSTM kernel for Trainium2, 8-core SPMD — v3 (dense normalized
adjacency streamed from host, interleaved 2-stream LSTM).

Reference semantics:
  1. GCN layer with symmetric normalization over a block-diagonal graph
     (200 graphs x 500 nodes, 1.6M edges), ReLU.
  2. Per-graph mean pooling -> [200, 128].
  3. Sliding windows (len 20) -> single-layer LSTM -> FC -> [181, 1].

Sharding: graph/data parallel. Core c owns graphs [25c, 25c+25). Per-graph
pooled embeddings are AllGather'd; the tiny LSTM is replicated on every core.

Host prep: the normalized adjacency  Â = D_out^-1/2 A D_in^-1/2  (the
canonical GCN preprocessing artifact) is built densely per graph [500, 500]
and shipped as fp8e4m3 in a src-window layout [128, 4, 500] per graph.
Mean-pool division by 500 is folded into w_gcn/b_gcn (relu is positively
homogeneous). Per graph:

  agg[64, 500]  = sum_a  x[:, 4g+a, :]^T @ Â_g[:, a, :]      (4 PE matmuls)
  aggs          = copy(agg)  fp16                            (Act)
  h3p[128, 500] = w_gcn^T @ aggs                             (1 PE matmul)
  pooled[:, g]  = sum_dst relu(h3p + b/500)                  (1 fused DVE op)

x is DMA'd FIRST so graph-0 compute overlaps the Â DMA stream. The LSTM
runs as two interleaved window streams (91+90) so the serial per-step
engine chain of one stream fills the other's stalls. Gates are permuted to
(i, f, o, g) so one strided Act call covers the three sigmoids; their
biases are accumulated into PSUM with a rank-1 matmul (ones ⊗ b), the tanh
gate takes its bias via the Act bias port.
"""

import numpy as np

# ---------------------------------------------------------------- constants
N_GRAPHS = 200
NPG = 500  # nodes per graph
DIN = 64
DGCN = 128
SEQ = 20
H = 128

N_CORES = 8
GPC = N_GRAPHS // N_CORES  # graphs per core: 25
P = 128
NSW = 4  # src windows per graph (128 wide)
NSLOT = GPC * NSW  # 100
B_WIN = N_GRAPHS - SEQ + 1  # 181
W0 = 91  # stream-0 windows
W1 = B_WIN - W0  # stream-1 windows


# ---------------------------------------------------------------- device IR
def build_nc(reps=1, gcn=True, cc=True, lstm=True, dma=True):
    import concourse.bacc as bacc
    import concourse.tile as tile
    import concourse.mybir as mybir

    f32 = mybir.dt.float32
    f16 = mybir.dt.float16
    f8 = mybir.dt.float8e4
    ALU = mybir.AluOpType
    ACT = mybir.ActivationFunctionType

    nc = bacc.Bacc(
        "TRN2",
        target_bir_lowering=False,
        debug=False,
        num_devices=N_CORES,
    )

    # inputs
    x_in = nc.dram_tensor("x", [P * NSLOT, DIN], f16, kind="ExternalInput").ap()
    a_in = nc.dram_tensor("a_hat", [GPC * P, NSW * NPG], f8, kind="ExternalInput").ap()
    w_gcn_in = nc.dram_tensor("w_gcn", [DIN, DGCN], f16, kind="ExternalInput").ap()
    b_gcn_in = nc.dram_tensor("b_gcn", [DGCN, 1], f32, kind="ExternalInput").ap()
    # LSTM weights with gate order permuted to (i, f, o, g)
    w_ihT_in = nc.dram_tensor("w_ihT", [DGCN, 4 * H], f16, kind="ExternalInput").ap()
    w_hhT_in = nc.dram_tensor("w_hhT", [H, 4 * H], f16, kind="ExternalInput").ap()
    b_row_in = nc.dram_tensor("b_row", [1, 4 * H], f16, kind="ExternalInput").ap()
    b_g_in = nc.dram_tensor("b_g", [H, 1], f32, kind="ExternalInput").ap()
    w_fcT_in = nc.dram_tensor("w_fcT", [H, 1], f16, kind="ExternalInput").ap()
    b_fc_in = nc.dram_tensor("b_fc", [1, 1], f32, kind="ExternalInput").ap()
    pred_out = nc.dram_tensor("pred", [1, B_WIN], f32, kind="ExternalOutput").ap()

    with tile.TileContext(nc) as tc:
        with (
            tc.tile_pool(name="dram", bufs=1, space="DRAM") as dpool,
            tc.tile_pool(name="const", bufs=1) as cpool,
            tc.tile_pool(name="work", bufs=3) as wpool,
            tc.tile_pool(name="pagg", bufs=2, space="PSUM") as pagg,
            tc.tile_pool(name="pproj", bufs=2, space="PSUM") as pproj,
            tc.tile_pool(name="pgate", bufs=2, space="PSUM") as pgate,
        ):
            # persistent tiles (re-DMA'd every rep)
            x_sb = cpool.tile([P, NSLOT, DIN], f16)
            a_sb = [
                cpool.tile([P, NSW, NPG], f8, name=f"a_sb{g}") for g in range(GPC)
            ]
            w_gcn_t = cpool.tile([DIN, DGCN], f16)
            b_gcn_t = cpool.tile([DGCN, 1], f32)
            w_ihT_t = cpool.tile([DGCN, 4 * H], f16)
            w_hhT_t = cpool.tile([H, 4 * H], f16)
            b_row_t = cpool.tile([1, 4 * H], f16)
            b_g_t = cpool.tile([H, 1], f32)
            w_fcT_t = cpool.tile([H, 1], f16)
            b_fc_t = cpool.tile([1, 1], f32)
            zeros_t = cpool.tile([P, 1], f32)
            ones_t = cpool.tile([1, B_WIN], f16)
            cc_in = dpool.tile([P, GPC], f32)
            cc_out = dpool.tile([P * N_CORES, GPC], f32)

            for _ in range(reps):
                # ---------------- load inputs (x first: graph-0 compute
                # overlaps the long Â stream)
                if dma:
                    xr = x_in[:].rearrange("(p s) d -> p s d", p=P)
                    half = NSLOT // 2
                    nc.sync.dma_start(x_sb[:, 0:half, :], xr[:, 0:half, :])
                    nc.sync.dma_start(x_sb[:, half:, :], xr[:, half:, :])
                nc.sync.dma_start(w_gcn_t[:], w_gcn_in[:])
                nc.sync.dma_start(b_gcn_t[:], b_gcn_in[:])
                if dma:
                    for g in range(GPC):
                        nc.sync.dma_start(
                            a_sb[g][:],
                            a_in[g * P : (g + 1) * P, :].rearrange(
                                "p (a d) -> p a d", a=NSW
                            ),
                        )
                nc.sync.dma_start(w_ihT_t[:], w_ihT_in[:])
                nc.sync.dma_start(w_hhT_t[:], w_hhT_in[:])
                nc.sync.dma_start(b_row_t[:], b_row_in[:])
                nc.sync.dma_start(b_g_t[:], b_g_in[:])
                nc.sync.dma_start(w_fcT_t[:], w_fcT_in[:])
                nc.sync.dma_start(b_fc_t[:], b_fc_in[:])
                nc.vector.memset(zeros_t[:], 0.0)
                nc.vector.memset(ones_t[:], 1.0)

                pooledT = wpool.tile([P, GPC], f32, tag="pooled")
                if not gcn:
                    nc.vector.memset(pooledT[:], 0.0)

                # ---------------- per-graph GCN
                for g in range(GPC if gcn else 0):
                    aggp = pagg.tile([DIN, NPG], f32, tag="agg")
                    for a in range(NSW):
                        nc.tensor.matmul(
                            aggp[:],
                            x_sb[:, g * NSW + a, :],
                            a_sb[g][:, a, :],
                            start=(a == 0),
                            stop=(a == NSW - 1),
                        )
                    aggs = wpool.tile([DIN, NPG], f16, tag="aggs")
                    nc.scalar.copy(aggs[:], aggp[:])
                    h3p = pproj.tile([DGCN, NPG], f32, tag="proj")
                    nc.tensor.matmul(
                        h3p[:], w_gcn_t[:], aggs[:], start=True, stop=True
                    )
                    h3r = wpool.tile([DGCN, NPG], f16, tag="h3r")
                    nc.vector.scalar_tensor_tensor(
                        h3r[:],
                        h3p[:],
                        b_gcn_t[:],
                        zeros_t[:].to_broadcast([DGCN, NPG]),
                        ALU.add,
                        ALU.max,
                        accum_out=pooledT[:, g : g + 1],
                    )

                # ---------------- all-gather pooled embeddings
                hgT16 = wpool.tile([P, N_GRAPHS], f16, tag="hgT16")
                if cc:
                    nc.sync.dma_start(cc_in[:], pooledT[:])
                    nc.gpsimd.collective_compute(
                        "AllGather",
                        ALU.bypass,
                        replica_groups=[list(range(N_CORES))],
                        ins=[cc_in.opt()],
                        outs=[cc_out.opt()],
                    )
                    hgT = wpool.tile([P, N_GRAPHS], f32, tag="hgT")
                    nc.sync.dma_start(
                        hgT[:].rearrange("p (c g) -> p c g", c=N_CORES),
                        cc_out[:].rearrange("(c p) g -> p c g", p=P),
                    )
                    nc.vector.tensor_copy(hgT16[:], hgT[:])
                else:
                    nc.vector.tensor_copy(hgT16[:, 0:GPC], pooledT[:])
                    nc.vector.memset(hgT16[:, GPC:], 0.0)

                if not lstm:
                    pred_t = wpool.tile([1, B_WIN], f32, tag="predt")
                    nc.vector.memset(pred_t[:], 0.0)
                    nc.vector.tensor_tensor(
                        pred_t[:, 0:1], pooledT[0:1, 0:1], hgT16[0:1, 0:1], ALU.add
                    )
                    nc.sync.dma_start(pred_out[:], pred_t[:])
                    continue

                # ---------------- LSTM: two interleaved window streams so
                # one stream's ops fill the other's dependency stalls.
                # Gate order (f, i, o, g); per-stream psum tile
                # [128, 4, 128] = 1 bank, gate k in [:, k, 0:W]. σ covers
                # f/i/o in one strided call; their biases come via a rank-1
                # (ones ⊗ b) matmul, tanh(g)'s via the Act bias port.
                cT = [
                    wpool.tile([H, W0], f32, tag="cT0", name="cT0"),
                    wpool.tile([H, W1], f32, tag="cT1", name="cT1"),
                ]
                hT16 = [
                    wpool.tile([H, W0], f16, tag="hT0", name="hT0"),
                    wpool.tile([H, W1], f16, tag="hT1", name="hT1"),
                ]
                streams = [(0, 0, W0), (1, W0, W1)]
                for l in range(SEQ):
                    for s, col0, W in streams:
                        gp = pgate.tile([H, 4, P], f32, tag=f"gp{s}")
                        for k in range(4):
                            gk = gp[:, k, 0:W]
                            nc.tensor.matmul(
                                gk,
                                w_ihT_t[:, k * H : (k + 1) * H],
                                hgT16[:, l + col0 : l + col0 + W],
                                start=True,
                                stop=(l == 0 and k == 3),
                            )
                            if k < 3:
                                nc.tensor.matmul(
                                    gk,
                                    b_row_t[:, k * H : (k + 1) * H],
                                    ones_t[:, 0:W],
                                    start=False,
                                    stop=(l == 0),
                                )
                            if l > 0:
                                nc.tensor.matmul(
                                    gk,
                                    w_hhT_t[:, k * H : (k + 1) * H],
                                    hT16[s][:],
                                    start=False,
                                    stop=True,
                                )
                        sg = wpool.tile([H, 3, W], f16, tag=f"sg{s}")
                        nc.scalar.activation(sg[:], gp[:, 0:3, 0:W], ACT.Sigmoid)
                        tg = wpool.tile([H, W], f16, tag=f"tg{s}")
                        nc.scalar.activation(
                            tg[:], gp[:, 3, 0:W], ACT.Tanh, bias=b_g_t[:]
                        )
                        if l == 0:
                            nc.vector.tensor_tensor(
                                cT[s][:], sg[:, 1, :], tg[:], ALU.mult
                            )
                        else:
                            t2 = wpool.tile([H, W], f32, tag=f"t2{s}")
                            nc.gpsimd.tensor_tensor(
                                t2[:], sg[:, 1, :], tg[:], ALU.mult
                            )
                            t1 = wpool.tile([H, W], f32, tag=f"t1{s}")
                            nc.vector.tensor_tensor(
                                t1[:], sg[:, 0, :], cT[s][:], ALU.mult
                            )
                            nc.vector.tensor_tensor(cT[s][:], t1[:], t2[:], ALU.add)
                        tch = wpool.tile([H, W], f16, tag=f"tch{s}")
                        nc.scalar.activation(tch[:], cT[s][:], ACT.Tanh)
                        nc.vector.tensor_tensor(
                            hT16[s][:], sg[:, 2, :], tch[:], ALU.mult
                        )

                pr = pagg.tile([1, B_WIN], f32, tag="agg")
                for s, col0, W in streams:
                    nc.tensor.matmul(
                        pr[:, col0 : col0 + W],
                        w_fcT_t[:],
                        hT16[s][:],
                        start=True,
                        stop=True,
                    )
                pred_t = wpool.tile([1, B_WIN], f32, tag="predt")
                nc.scalar.activation(pred_t[:], pr[:], ACT.Identity, bias=b_fc_t[:])
                nc.sync.dma_start(pred_out[:], pred_t[:])

    nc.compile()
    return nc


# ---------------------------------------------------------------- host prep
def make_in_maps(x, src, dst, w_gcn, b_gcn, w_ih, w_hh, b_ih, b_hh, w_fc, b_fc):
    import ml_dtypes

    f8 = ml_dtypes.float8_e4m3
    src = np.asarray(src).astype(np.int64)
    dst = np.asarray(dst).astype(np.int64)
    x = np.asarray(x, np.float32)

    # normalized adjacency per graph: Â = D_out^-1/2 A D_in^-1/2
    dl = dst % NPG
    counts = np.bincount(src * NPG + dl, minlength=N_GRAPHS * NPG * NPG)
    A = counts.astype(np.float32).reshape(N_GRAPHS, NPG, NPG)
    od = np.maximum(A.sum(2), 1.0) ** -0.5  # [G, src]
    idg = np.maximum(A.sum(1), 1.0) ** -0.5  # [G, dst]
    A *= od[:, :, None]
    A *= idg[:, None, :]
    # pad src dim 500 -> 512, window layout [G, 128, 4, 500], fp8
    Ap = np.zeros((N_GRAPHS, NSW * P, NPG), np.float32)
    Ap[:, :NPG, :] = A
    A8 = (
        Ap.reshape(N_GRAPHS, NSW, P, NPG)
        .transpose(0, 2, 1, 3)
        .reshape(N_GRAPHS, P, NSW * NPG)
        .astype(f8)
    )

    # gate order (f, i, o, g): PyTorch rows [0:H]=i [H:2H]=f [2H:3H]=g [3H:4H]=o
    perm = np.concatenate(
        [np.arange(H, 2 * H), np.arange(H), np.arange(3 * H, 4 * H),
         np.arange(2 * H, 3 * H)]
    )
    w_ihp = np.asarray(w_ih)[perm]
    w_hhp = np.asarray(w_hh)[perm]
    b_comb = (np.asarray(b_ih) + np.asarray(b_hh))[perm]

    common = {
        "w_gcn": np.ascontiguousarray((np.asarray(w_gcn) / NPG).astype(np.float16)),
        "b_gcn": np.ascontiguousarray(
            (np.asarray(b_gcn) / NPG).astype(np.float32).reshape(DGCN, 1)
        ),
        "w_ihT": np.ascontiguousarray(w_ihp.T.astype(np.float16)),
        "w_hhT": np.ascontiguousarray(w_hhp.T.astype(np.float16)),
        "b_row": np.ascontiguousarray(b_comb.astype(np.float16).reshape(1, 4 * H)),
        "b_g": np.ascontiguousarray(
            b_comb[3 * H : 4 * H].astype(np.float32).reshape(H, 1)
        ),
        "w_fcT": np.ascontiguousarray(np.asarray(w_fc).T.astype(np.float16)),
        "b_fc": np.ascontiguousarray(np.asarray(b_fc).astype(np.float32).reshape(1, 1)),
    }

    # x permuted to [p, slot, d]: node n = 500g + 128a + p, slot = 4g + a
    in_maps = []
    x16 = x.astype(np.float16)
    for c in range(N_CORES):
        xc = x16[c * GPC * NPG : (c + 1) * GPC * NPG].reshape(GPC, NPG, DIN)
        xp = np.zeros((P, NSLOT, DIN), np.float16)
        for a in range(NSW):
            base = P * a
            rows = min(NPG - base, P)
            xp[:rows, a::NSW, :] = xc[:, base : base + rows, :].transpose(1, 0, 2)
        ac = A8[c * GPC : (c + 1) * GPC]  # [25, 128, 2000]
        in_maps.append(
            {
                "x": np.ascontiguousarray(xp.reshape(P * NSLOT, DIN)),
                "a_hat": np.ascontiguousarray(ac.reshape(GPC * P, NSW * NPG)),
                **common,
            }
        )
    return in_maps


# ---------------------------------------------------------------- entry
_CACHE = {}


def kernel(x, src, dst, graph_ids, w_gcn, b_gcn, w_ih, w_hh, b_ih, b_hh, w_fc, b_fc):
    from concourse import bass_utils

    in_maps = make_in_maps(
        x, src, dst, w_gcn, b_gcn, w_ih, w_hh, b_ih, b_hh, w_fc, b_fc
    )
    if "nc" not in _CACHE:
        _CACHE["nc"] = build_nc(reps=1)
    nc = _CACHE["nc"]
    res = bass_utils.run_bass_kernel_spmd(
        nc, in_maps, core_ids=list(range(N_CORES))
    )
    pred = res.results[0]["pred"]  # [1, 181]
    return np.ascontiguousarray(pred.reshape(-1, 1).astype(np.float32))


# revision 15
# speedup vs baseline: 1.3561x; 1.3561x over previous
"""GCN + LSTM kernel for Trainium2, 8-core SPMD — v2 (dense normalized
adjacency streamed from host).

Reference semantics:
  1. GCN layer with symmetric normalization over a block-diagonal graph
     (200 graphs x 500 nodes, 1.6M edges), ReLU.
  2. Per-graph mean pooling -> [200, 128].
  3. Sliding windows (len 20) -> single-layer LSTM -> FC -> [181, 1].

Sharding: graph/data parallel. Core c owns graphs [25c, 25c+25). Per-graph
pooled embeddings are AllGather'd; the tiny LSTM is replicated on every core.

Host prep: the normalized adjacency  Â = D_out^-1/2 A D_in^-1/2  (the
canonical GCN preprocessing artifact) is built densely per graph [500, 500]
and shipped as fp8e4m3 in a src-window layout [128, 4, 500] per graph.
Mean-pool division by 500 is folded into w_gcn/b_gcn (relu is positively
homogeneous), so on device each graph is:

  agg[64, 500]  = sum_a  x[:, 4g+a, :]^T @ Â_g[:, a, :]      (4 PE matmuls)
  aggs          = copy(agg)  fp16                            (Act)
  h3p[128, 500] = w_gcn^T @ aggs                             (1 PE matmul)
  pooled[:, g]  = sum_dst relu(h3p + b/500)                  (1 fused DVE op)

followed by AllGather of pooled [128, 25] and a replicated 20-step LSTM with
the x-projection accumulated directly in PSUM (w_ih@hg then w_hh@h into the
same bank), gate activations on the Act engine, and the c/h chain on DVE.
"""

import numpy as np

# ---------------------------------------------------------------- constants
N_GRAPHS = 200
NPG = 500  # nodes per graph
DIN = 64
DGCN = 128
SEQ = 20
H = 128

N_CORES = 8
GPC = N_GRAPHS // N_CORES  # graphs per core: 25
P = 128
NSW = 4  # src windows per graph (128 wide)
NSLOT = GPC * NSW  # 100
B_WIN = N_GRAPHS - SEQ + 1  # 181


# ---------------------------------------------------------------- device IR
def build_nc(reps=1):
    import concourse.bacc as bacc
    import concourse.tile as tile
    import concourse.mybir as mybir

    f32 = mybir.dt.float32
    f16 = mybir.dt.float16
    f8 = mybir.dt.float8e4
    ALU = mybir.AluOpType
    ACT = mybir.ActivationFunctionType

    nc = bacc.Bacc(
        "TRN2",
        target_bir_lowering=False,
        debug=False,
        num_devices=N_CORES,
    )

    # inputs
    x_in = nc.dram_tensor("x", [P * NSLOT, DIN], f16, kind="ExternalInput").ap()
    a_in = nc.dram_tensor("a_hat", [GPC * P, NSW * NPG], f8, kind="ExternalInput").ap()
    w_gcn_in = nc.dram_tensor("w_gcn", [DIN, DGCN], f16, kind="ExternalInput").ap()
    b_gcn_in = nc.dram_tensor("b_gcn", [DGCN, 1], f32, kind="ExternalInput").ap()
    w_ihT_in = nc.dram_tensor("w_ihT", [DGCN, 4 * H], f16, kind="ExternalInput").ap()
    w_hhT_in = nc.dram_tensor("w_hhT", [H, 4 * H], f16, kind="ExternalInput").ap()
    b_comb_in = nc.dram_tensor("b_comb", [H, 4], f32, kind="ExternalInput").ap()
    w_fcT_in = nc.dram_tensor("w_fcT", [H, 1], f16, kind="ExternalInput").ap()
    b_fc_in = nc.dram_tensor("b_fc", [1, 1], f32, kind="ExternalInput").ap()
    pred_out = nc.dram_tensor("pred", [1, B_WIN], f32, kind="ExternalOutput").ap()

    act_of = {0: ACT.Sigmoid, 1: ACT.Sigmoid, 2: ACT.Tanh, 3: ACT.Sigmoid}

    with tile.TileContext(nc) as tc:
        with (
            tc.tile_pool(name="dram", bufs=1, space="DRAM") as dpool,
            tc.tile_pool(name="const", bufs=1) as cpool,
            tc.tile_pool(name="work", bufs=3) as wpool,
            tc.tile_pool(name="pagg", bufs=2, space="PSUM") as pagg,
            tc.tile_pool(name="pproj", bufs=2, space="PSUM") as pproj,
            tc.tile_pool(name="pgate", bufs=1, space="PSUM") as pgate,
        ):
            # persistent tiles (re-DMA'd every rep)
            x_sb = cpool.tile([P, NSLOT, DIN], f16)
            a_sb = [
                cpool.tile([P, NSW, NPG], f8, name=f"a_sb{g}") for g in range(GPC)
            ]
            w_gcn_t = cpool.tile([DIN, DGCN], f16)
            b_gcn_t = cpool.tile([DGCN, 1], f32)
            w_ihT_t = cpool.tile([DGCN, 4 * H], f16)
            w_hhT_t = cpool.tile([H, 4 * H], f16)
            b_comb_t = cpool.tile([H, 4], f32)
            w_fcT_t = cpool.tile([H, 1], f16)
            b_fc_t = cpool.tile([1, 1], f32)
            zeros_t = cpool.tile([P, 1], f32)
            cc_in = dpool.tile([P, GPC], f32)
            cc_out = dpool.tile([P * N_CORES, GPC], f32)

            for _ in range(reps):
                # ---------------- load inputs (x + GCN weights first so
                # graph-0 compute overlaps the long Â DMA stream)
                xr = x_in[:].rearrange("(p s) d -> p s d", p=P)
                half = NSLOT // 2
                nc.sync.dma_start(x_sb[:, 0:half, :], xr[:, 0:half, :])
                nc.sync.dma_start(x_sb[:, half:, :], xr[:, half:, :])
                nc.sync.dma_start(w_gcn_t[:], w_gcn_in[:])
                nc.sync.dma_start(b_gcn_t[:], b_gcn_in[:])
                for g in range(GPC):
                    nc.sync.dma_start(
                        a_sb[g][:],
                        a_in[g * P : (g + 1) * P, :].rearrange(
                            "p (a d) -> p a d", a=NSW
                        ),
                    )
                nc.sync.dma_start(w_ihT_t[:], w_ihT_in[:])
                nc.sync.dma_start(w_hhT_t[:], w_hhT_in[:])
                nc.sync.dma_start(b_comb_t[:], b_comb_in[:])
                nc.sync.dma_start(w_fcT_t[:], w_fcT_in[:])
                nc.sync.dma_start(b_fc_t[:], b_fc_in[:])
                nc.vector.memset(zeros_t[:], 0.0)

                pooledT = wpool.tile([P, GPC], f32, tag="pooled")

                # ---------------- per-graph GCN
                for g in range(GPC):
                    aggp = pagg.tile([DIN, NPG], f32, tag="agg")
                    for a in range(NSW):
                        nc.tensor.matmul(
                            aggp[:],
                            x_sb[:, g * NSW + a, :],
                            a_sb[g][:, a, :],
                            start=(a == 0),
                            stop=(a == NSW - 1),
                        )
                    aggs = wpool.tile([DIN, NPG], f16, tag="aggs")
                    nc.scalar.copy(aggs[:], aggp[:])
                    h3p = pproj.tile([DGCN, NPG], f32, tag="proj")
                    nc.tensor.matmul(
                        h3p[:], w_gcn_t[:], aggs[:], start=True, stop=True
                    )
                    h3r = wpool.tile([DGCN, NPG], f16, tag="h3r")
                    nc.vector.scalar_tensor_tensor(
                        h3r[:],
                        h3p[:],
                        b_gcn_t[:],
                        zeros_t[:].to_broadcast([DGCN, NPG]),
                        ALU.add,
                        ALU.max,
                        accum_out=pooledT[:, g : g + 1],
                    )

                # ---------------- all-gather pooled embeddings
                nc.sync.dma_start(cc_in[:], pooledT[:])
                nc.gpsimd.collective_compute(
                    "AllGather",
                    ALU.bypass,
                    replica_groups=[list(range(N_CORES))],
                    ins=[cc_in.opt()],
                    outs=[cc_out.opt()],
                )
                hgT = wpool.tile([P, N_GRAPHS], f32, tag="hgT")
                nc.sync.dma_start(
                    hgT[:].rearrange("p (c g) -> p c g", c=N_CORES),
                    cc_out[:].rearrange("(c p) g -> p c g", p=P),
                )
                hgT16 = wpool.tile([P, N_GRAPHS], f16, tag="hgT16")
                nc.vector.tensor_copy(hgT16[:], hgT[:])

                # ---------------- LSTM (PyTorch gate order i,f,g,o)
                cT = wpool.tile([H, B_WIN], f32, tag="cT")
                hT16 = wpool.tile([H, B_WIN], f16, tag="hT16")
                for l in range(SEQ):
                    gate = []
                    for k in range(4):
                        gp = pgate.tile([H, B_WIN], f32, tag=f"g{k}")
                        nc.tensor.matmul(
                            gp[:],
                            w_ihT_t[:, k * H : (k + 1) * H],
                            hgT16[:, l : l + B_WIN],
                            start=True,
                            stop=(l == 0),
                        )
                        if l > 0:
                            nc.tensor.matmul(
                                gp[:],
                                w_hhT_t[:, k * H : (k + 1) * H],
                                hT16[:],
                                start=False,
                                stop=True,
                            )
                        ga = wpool.tile([H, B_WIN], f16, tag=f"ga{k}")
                        nc.scalar.activation(
                            ga[:], gp[:], act_of[k], bias=b_comb_t[:, k : k + 1]
                        )
                        gate.append(ga)
                    t2 = wpool.tile([H, B_WIN], f32, tag="t2")
                    nc.vector.tensor_tensor(t2[:], gate[0][:], gate[2][:], ALU.mult)
                    if l == 0:
                        nc.vector.tensor_copy(cT[:], t2[:])
                    else:
                        t1 = wpool.tile([H, B_WIN], f32, tag="t1")
                        nc.vector.tensor_tensor(t1[:], gate[1][:], cT[:], ALU.mult)
                        nc.vector.tensor_tensor(cT[:], t1[:], t2[:], ALU.add)
                    tch = wpool.tile([H, B_WIN], f16, tag="tch")
                    nc.scalar.activation(tch[:], cT[:], ACT.Tanh)
                    nc.vector.tensor_tensor(hT16[:], gate[3][:], tch[:], ALU.mult)

                pr = pagg.tile([1, B_WIN], f32, tag="agg")
                nc.tensor.matmul(pr[:], w_fcT_t[:], hT16[:], start=True, stop=True)
                pred_t = wpool.tile([1, B_WIN], f32, tag="predt")
                nc.scalar.activation(pred_t[:], pr[:], ACT.Identity, bias=b_fc_t[:])
                nc.sync.dma_start(pred_out[:], pred_t[:])

    nc.compile()
    return nc


# ---------------------------------------------------------------- host prep
def make_in_maps(x, src, dst, w_gcn, b_gcn, w_ih, w_hh, b_ih, b_hh, w_fc, b_fc):
    import ml_dtypes

    f8 = ml_dtypes.float8_e4m3
    src = np.asarray(src).astype(np.int64)
    dst = np.asarray(dst).astype(np.int64)
    x = np.asarray(x, np.float32)

    # normalized adjacency per graph: Â = D_out^-1/2 A D_in^-1/2
    dl = dst % NPG
    counts = np.bincount(src * NPG + dl, minlength=N_GRAPHS * NPG * NPG)
    A = counts.astype(np.float32).reshape(N_GRAPHS, NPG, NPG)
    od = np.maximum(A.sum(2), 1.0) ** -0.5  # [G, src]
    idg = np.maximum(A.sum(1), 1.0) ** -0.5  # [G, dst]
    A *= od[:, :, None]
    A *= idg[:, None, :]
    # pad src dim 500 -> 512, window layout [G, 128, 4, 500], fp8
    Ap = np.zeros((N_GRAPHS, NSW * P, NPG), np.float32)
    Ap[:, :NPG, :] = A
    A8 = (
        Ap.reshape(N_GRAPHS, NSW, P, NPG)
        .transpose(0, 2, 1, 3)
        .reshape(N_GRAPHS, P, NSW * NPG)
        .astype(f8)
    )

    common = {
        "w_gcn": np.ascontiguousarray((w_gcn / NPG).astype(np.float16)),
        "b_gcn": np.ascontiguousarray(
            (b_gcn / NPG).astype(np.float32).reshape(DGCN, 1)
        ),
        "w_ihT": np.ascontiguousarray(w_ih.T.astype(np.float16)),
        "w_hhT": np.ascontiguousarray(w_hh.T.astype(np.float16)),
        "b_comb": np.ascontiguousarray(
            (b_ih + b_hh).astype(np.float32).reshape(4, H).T
        ),
        "w_fcT": np.ascontiguousarray(w_fc.T.astype(np.float16)),
        "b_fc": np.ascontiguousarray(b_fc.astype(np.float32).reshape(1, 1)),
    }

    # x permuted to [p, slot, d]: node n = 500g + 128a + p, slot = 4g + a
    in_maps = []
    x16 = x.astype(np.float16)
    for c in range(N_CORES):
        xc = x16[c * GPC * NPG : (c + 1) * GPC * NPG].reshape(GPC, NPG, DIN)
        xp = np.zeros((P, NSLOT, DIN), np.float16)
        for a in range(NSW):
            base = P * a
            rows = min(NPG - base, P)
            xp[:rows, a::NSW, :] = xc[:, base : base + rows, :].transpose(1, 0, 2)
        ac = A8[c * GPC : (c + 1) * GPC]  # [25, 128, 2000]
        in_maps.append(
            {
                "x": np.ascontiguousarray(xp.reshape(P * NSLOT, DIN)),
                "a_hat": np.ascontiguousarray(ac.reshape(GPC * P, NSW * NPG)),
                **common,
            }
        )
    return in_maps


# ---------------------------------------------------------------- entry
_CACHE = {}


def kernel(x, src, dst, graph_ids, w_gcn, b_gcn, w_ih, w_hh, b_ih, b_hh, w_fc, b_fc):
    from concourse import bass_utils

    in_maps = make_in_maps(
        x, src, dst, w_gcn, b_gcn, w_ih, w_hh, b_ih, b_hh, w_fc, b_fc
    )
    if "nc" not in _CACHE:
        _CACHE["nc"] = build_nc(reps=1)
    nc = _CACHE["nc"]
    res = bass_utils.run_bass_kernel_spmd(
        nc, in_maps, core_ids=list(range(N_CORES))
    )
    pred = res.results[0]["pred"]  # [1, 181]
    return np.ascontiguousarray(pred.reshape(-1, 1).astype(np.float32))
